# revision 58
# baseline (speedup 1.0000x reference)
"""Trainium2 Bass kernel for nn_AttentionLayer (Bahdanau-style attention).

reference math (per batch b):
    query  = dec[b] @ W_q                      # [H]
    keys   = enc[b] @ W_k                      # [S, H]
    energy = tanh(query + keys)                # [S, H]
    scores = energy @ V                        # [S]
    attn   = softmax(scores)                   # [S]
    ctx    = attn @ enc[b]                     # [H]

Sharding: data-parallel over batch B=32 across 8 NeuronCores (4 batches/core),
weights replicated, no collectives.

Per-core dataflow (per batch):
  - enc rows are DMA-loaded natural [s,h] fp32 (HWDGE), cast to bf16 on DVE,
    and kept resident for the batch; each 128-row block is also transposed
    SBUF->SBUF by the DMA xbar (dma_start(transpose=True)) into encT tiles
    [128 h_p, 8 h_c, 512 s] so the keys GEMM contracts over h on partitions:
        keysT[j, s] = sum_h W_k[h, j] * encT[h, s]
    with W_k chunks [128h, 128j] stationary and encT [128h, 512s] moving
    -> PSUM [128j, 512s] fp32, accumulated over the 8 h-chunks.
  - tanh(keysT + query) is fused on ScalarE reading PSUM with a per-partition
    bias = queryT[:, j-chunk, b] (query broadcast along the s free dim).
  - scores come from a second matmul: lhsT = V chunk [128,1] bf16, accumulated
    over the 8 j-chunks into PSUM [1, 512].
  - softmax is fp32 on [1, S] rows (reduce_max(negate) -> Exp with bias and
    fused accum_out denominator -> reciprocal -> scale).
  - context = attn @ enc: attn is cast to bf16, partition-scattered and
    xbar-transposed to attnT [128 s, NSC]; the resident bf16 enc rows are the
    moving operand; PSUM accumulates over the 16 s-chunks.

The final module is post-processed by split_excess_waits(): walrus allows only
one sync-wait slot per ISA instruction, so surplus Tile-emitted waits are
hoisted into standalone InstEventSemaphore instructions on the same engine.
"""

import numpy as np
from contextlib import ExitStack

import concourse.bass as bass
import concourse.tile as tile
from concourse import mybir
from concourse.masks import make_identity
from concourse.bass_utils import run_bass_kernel_spmd

F32 = mybir.dt.float32
BF16 = mybir.dt.bfloat16
AF = mybir.ActivationFunctionType
AX = mybir.AxisListType
P = 128

B_FULL, S_FULL, H_FULL = 32, 2048, 1024
N_CORES = 8


def emit(ctx: ExitStack, tc: "tile.TileContext", enc, dec, wq, wk, v,
         ctx_out, attn_out, NB: int, S: int, H: int):
    nc = tc.nc
    HC = H // P            # h-chunks (contraction)
    JC = H // P            # j-chunks (keys output feature dim)
    ST = min(512, S)       # s-tile width (matmul moving dim / PSUM bank)
    NST = S // ST          # s-tiles per batch
    SB = ST // P           # 128-row s-blocks per s-tile
    NSC = S // P           # s-chunks per batch (context contraction)
    CW = min(512, H)       # context psum width
    NCW = H // CW

    const = ctx.enter_context(tc.tile_pool(name="const", bufs=1))
    wkp = ctx.enter_context(tc.tile_pool(name="wkp", bufs=1))
    fload = ctx.enter_context(tc.tile_pool(name="fload", bufs=6))
    enat_p = ctx.enter_context(tc.tile_pool(name="enat", bufs=2 * NSC))
    etp = ctx.enter_context(tc.tile_pool(name="et", bufs=2))
    eng_p = ctx.enter_context(tc.tile_pool(name="energy", bufs=2 * JC + 2))
    small = ctx.enter_context(tc.tile_pool(name="smallsb", bufs=1))
    scores_p = ctx.enter_context(tc.tile_pool(name="scoresp", bufs=2))

    ident_f = const.tile([P, P], F32)
    make_identity(nc, ident_f[:])

    scores_t = {}
    dens_t = {}
    w16_t = {}
    enat_tiles = {}

    def load_stile(b, t):
        """Load+cast one s-tile of enc (HWDGE fp32 + DVE bf16 cast) and
        xbar-transpose it into encT layout."""
        stile_en = []
        for sb in range(SB):
            scn = t * SB + sb
            tf = fload.tile([P, H], F32, tag="fload", name=f"ef{b}_{t}_{sb}")
            s0 = t * ST + sb * P
            nc.sync.dma_start(tf[:], enc[b, s0:s0 + P, :])
            en = enat_p.tile([P, H], BF16, tag="enat", name=f"en{b}_{scn}")
            nc.vector.tensor_copy(en[:], tf[:])
            enat_tiles[(b, scn)] = en
            stile_en.append(en)
        et = etp.tile([P, HC, ST], BF16, tag="et", name=f"et{b}_{t}")
        for sb in range(SB):
            nc.sync.dma_start(et[:, :, sb * P:(sb + 1) * P], stile_en[sb][:],
                              transpose=True)
        return et

    # ---------------- phase 0: weights / query ----------------
    # (scoped PSUM pool: phase-0 banks are released before the main pools)
    wkb = []
    queryT = const.tile([P, JC, NB], F32)
    with tc.tile_pool(name="ph0psum", bufs=1, space="PSUM") as ph0:
        # small inputs first so their DMAs aren't queued behind the weights
        dec_nat = const.tile([NB, H], F32)
        nc.sync.dma_start(dec_nat[:], dec[:, :])
        v_nat = const.tile([1, H], F32)
        nc.sync.dma_start(v_nat[:], v[None, :])
        v_b = const.tile([1, H], BF16)
        nc.vector.tensor_copy(v_b[:], v_nat[:])
        HCp = max(HC, 16)
        v_sc = const.tile([HCp, P], BF16)
        if HCp != HC:
            nc.vector.memset(v_sc[:], 0.0)
        nc.sync.dma_start(v_sc[:HC, :], v_b[:])
        vt_full = const.tile([P, HCp], BF16)
        nc.sync.dma_start(vt_full[:], v_sc[:], transpose=True)
        vt = vt_full[:, :HC]

        # dec -> decT [128 h, HC, NB] f32 (small PE transposes; f32 precision)
        # PE warm-up dummy: observe the gpsimd identity sem with a 1-dep op
        pdum = ph0.tile([P, 4], F32, tag="small", name="pdum")
        nc.tensor.transpose(pdum[:4, :4], ident_f[:4, :4], ident_f[:4, :4])
        decT = const.tile([P, HC, NB], F32)
        for c in range(HC):
            pd = ph0.tile([P, 4], F32, tag="small", name=f"pdec{c}")
            nc.tensor.transpose(pd[:, :NB], dec_nat[:, c * P:(c + 1) * P],
                                ident_f[:NB, :NB])
            nc.vector.tensor_copy(decT[:, c, :], pd[:, :NB])

        # query in natural layout: q_nat[b, j] accumulated over h-chunks with
        # bf16 matmuls (lhsT = decT chunk [128,NB], rhs = W_q rows, fp32
        # accumulation in PSUM), then transposed into queryT [128 j, JC, NB].
        decTb = const.tile([P, HC, NB], BF16)
        nc.vector.tensor_copy(decTb[:], decT[:])
        q_nat_ps = ph0.tile([NB, H], F32, tag="qnat")
        for c in range(HC):
            tf = scores_p.tile([P, H], BF16, tag="wqb", name=f"wqf{c}")
            nc.gpsimd.dma_start(tf[:], wq[c * P:(c + 1) * P, :])
            for half in range(H // CW):
                nc.tensor.matmul(q_nat_ps[:, half * CW:(half + 1) * CW],
                                 decTb[:, c, :], tf[:, half * CW:(half + 1) * CW],
                                 start=(c == 0), stop=(c == HC - 1))
        q_nat = const.tile([NB, H], F32)
        nc.vector.tensor_copy(q_nat[:], q_nat_ps[:])
        for c in range(JC):
            pqt = ph0.tile([P, 4], F32, tag="small", name=f"pqt{c}")
            nc.tensor.transpose(pqt[:, :NB], q_nat[:, c * P:(c + 1) * P],
                                ident_f[:NB, :NB])
            nc.vector.tensor_copy(queryT[:, c, :], pqt[:, :NB])

        # W_k after W_q: needed later (first K-matmul), loads overlap query.
        # SWDGE in-flight cast (gpsimd is idle here; halves DMA write bytes).
        for c in range(HC):
            tb = wkp.tile([P, H], BF16, tag=f"wk{c}")
            nc.gpsimd.dma_start(tb[:], wk[c * P:(c + 1) * P, :])
            wkb.append(tb)

    psum_k = ctx.enter_context(tc.tile_pool(name="psk", bufs=4, space="PSUM"))
    psum_s = ctx.enter_context(tc.tile_pool(name="pss", bufs=2, space="PSUM"))
    psum_c = ctx.enter_context(tc.tile_pool(name="psc", bufs=2, space="PSUM"))

    # ---------------- main loop ----------------
    pending = []  # [(b, t, [energy tiles])]

    def flush_pending():
        # V-matmuls for a finished s-tile, then incremental softmax pieces.
        # No max subtraction: |scores| <= sum|V| (tanh in [-1,1]) stays far
        # below fp32/bf16 exp overflow, so exp(s) is computed per s-tile as
        # soon as its scores land, with per-tile partial denominators.
        while pending:
            b, t, eys = pending.pop(0)
            ps = psum_s.tile([1, ST], F32, tag="ps")
            for jc in range(JC):
                nc.tensor.matmul(ps[:], vt[:, jc:jc + 1], eys[jc][:],
                                 start=(jc == 0), stop=(jc == JC - 1))
            nc.any.tensor_copy(scores_t[b][:, t * ST:(t + 1) * ST], ps[:])
            # incremental per-tile max on DVE (keeps the tail reduction tiny)
            nc.vector.reduce_max(dens_t[b][:, t:t + 1],
                                 scores_t[b][:, t * ST:(t + 1) * ST], axis=AX.X)

    def tail(b):
        # softmax: the 1/denominator scale is deferred -- attnT is built from
        # the unnormalized exp; the scale folds into the context eviction.
        srow = scores_t[b][:]
        mneg = small.tile([1, 1], F32, tag="mneg")
        nc.vector.reduce_max(mneg[:], dens_t[b][:], axis=AX.X, negate=True)
        # bf16 exp straight onto the attnT critical path
        arow_b = small.tile([1, S], BF16, tag="arow_b")
        den = small.tile([1, 1], F32, tag="den")
        nc.scalar.activation(arow_b[:], srow[:], AF.Exp, bias=mneg[:],
                             accum_out=den[:])
        nc.gpsimd.dma_start(w16_t[b][:NSC, :], arow_b[:])
        wT_full = small.tile([P, NSCp], BF16, tag="wT")
        nc.sync.dma_start(wT_full[:], w16_t[b][:], transpose=True)
        wT = wT_full[:, :NSC]

        rec = small.tile([1, 1], F32, tag="rec")
        nc.vector.reciprocal(rec[:], den[:])

        # attn output (normalized, fp32 exp), off the PE critical path
        arow = small.tile([1, S], F32, tag="arow")
        nc.scalar.activation(arow[:], srow[:], AF.Exp, bias=mneg[:])
        arow_n = small.tile([1, S], F32, tag="arow_n")
        nc.vector.tensor_scalar_mul(arow_n[:], arow[:], rec[:])
        nc.sync.dma_start(attn_out[b:b + 1, :], arow_n[:])

        # context: ctx[h] = sum_s attn[s] enc[s, h] over resident bf16 tiles
        pcs = [psum_c.tile([1, CW], F32, tag="pc", name=f"pc{b}_{i}")
               for i in range(NCW)]
        for half in range(NCW):
            for scn in range(NSC):
                nc.tensor.matmul(pcs[half][:], wT[:, scn:scn + 1],
                                 enat_tiles[(b, scn)][:, half * CW:(half + 1) * CW],
                                 start=(scn == 0), stop=(scn == NSC - 1))
        cxt = small.tile([1, H], F32, tag="cxt")
        for half in range(NCW):
            nc.scalar.activation(cxt[:, half * CW:(half + 1) * CW], pcs[half][:],
                                 AF.Copy, scale=rec[:])
        nc.sync.dma_start(ctx_out[b:b + 1, :], cxt[:])

    tail_t = min(2, NST - 1)
    NSCp = max(NSC, 16)
    for b in range(NB):
        scores_t[b] = scores_p.tile([1, S], F32, tag="scores",
                                    name=f"scores{b}")
        dens_t[b] = scores_p.tile([1, NST], F32, tag="dens",
                                  name=f"dens{b}")
        w16_t[b] = scores_p.tile([NSCp, P], BF16, tag="w16",
                                 name=f"w16_{b}")
        if NSCp != NSC:
            nc.vector.memset(w16_t[b][:], 0.0)
        for t in range(NST):
            et = load_stile(b, t)
            # keys GEMM + fused tanh
            eys = []
            for jc in range(JC):
                pk = psum_k.tile([P, ST], F32, tag="pk")
                for c in range(HC):
                    nc.tensor.matmul(pk[:], wkb[c][:, jc * P:(jc + 1) * P],
                                     et[:, c, :], start=(c == 0), stop=(c == HC - 1))
                ey = eng_p.tile([P, ST], BF16, tag="ey")
                nc.scalar.activation(ey[:], pk[:], AF.Tanh,
                                     bias=queryT[:, jc, b:b + 1])
                eys.append(ey)
            # deferred V-matmuls of the previous s-tile (keeps PE dense)
            flush_pending()
            pending.append((b, t, eys))
            if t == tail_t and b > 0:
                tail(b - 1)
    flush_pending()
    tail(NB - 1)


_NO_SPLIT = {"InstEventSemaphore", "InstNoOp",
             "InstUnconditionalBranch", "InstHalt"}


def split_excess_waits(nc) -> int:
    """Walrus ISA instruction encodings have a single sync-wait slot. Tile can
    emit more. Hoist the excess into standalone InstEventSemaphore
    instructions on the same engine, immediately before the instruction."""
    n_split = 0
    for f in nc.m.functions:
        for blk in f.blocks:
            insts = blk.instructions
            out = []
            for inst in insts:
                ty = type(inst).__name__
                si = inst.sync_info
                if si is None or ty in _NO_SPLIT:
                    out.append(inst)
                    continue
                limit = 1
                waits = list(si.on_wait)
                if len(waits) > limit:
                    excess, keep = waits[:-limit], waits[-limit:]
                    for w in excess:
                        n_split += 1
                        ev = mybir.InstEventSemaphore(
                            name=f"I-evw{n_split}", ins=[], outs=[],
                            engine=inst.engine,
                            sync_info=mybir.SyncInfo(on_wait=[w], on_update=[]),
                            bass_nofuse=True,
                        )
                        out.append(ev)
                    inst.sync_info = mybir.SyncInfo(on_wait=keep,
                                                   on_update=list(si.on_update))
                out.append(inst)
            insts[:] = out
    return n_split


def build_nc(NB: int, S: int, H: int, split_waits: bool = True) -> bass.Bass:
    nc = bass.Bass("TRN2", target_bir_lowering=False, debug=False)
    enc = nc.dram_tensor("enc", [NB, S, H], F32, kind="ExternalInput").ap()
    dec = nc.dram_tensor("dec", [NB, H], F32, kind="ExternalInput").ap()
    wq = nc.dram_tensor("wq", [H, H], F32, kind="ExternalInput").ap()
    wk = nc.dram_tensor("wk", [H, H], F32, kind="ExternalInput").ap()
    v = nc.dram_tensor("v", [H], F32, kind="ExternalInput").ap()
    ctx_out = nc.dram_tensor("ctx", [NB, H], F32, kind="ExternalOutput").ap()
    attn_out = nc.dram_tensor("attn", [NB, S], F32, kind="ExternalOutput").ap()
    with tile.TileContext(nc) as tc, ExitStack() as ctx:
        emit(ctx, tc, enc, dec, wq, wk, v, ctx_out, attn_out, NB, S, H)
    if split_waits:
        split_excess_waits(nc)
    return nc


_NC_CACHE = {}


def get_nc(NB: int, S: int, H: int) -> bass.Bass:
    key = (NB, S, H)
    if key not in _NC_CACHE:
        _NC_CACHE[key] = build_nc(NB, S, H)
    return _NC_CACHE[key]


def make_in_maps(enc, dec, wq, wk, v, n_cores):
    nb = enc.shape[0] // n_cores
    return [
        {
            "enc": np.ascontiguousarray(enc[i * nb:(i + 1) * nb]),
            "dec": np.ascontiguousarray(dec[i * nb:(i + 1) * nb]),
            "wq": wq,
            "wk": wk,
            "v": v,
        }
        for i in range(n_cores)
    ]


def kernel(**inputs) -> tuple:
    enc = np.asarray(inputs["encoder_outputs"], dtype=np.float32)
    dec = np.asarray(inputs["decoder_hidden"], dtype=np.float32)
    wq = np.ascontiguousarray(np.asarray(inputs["W_query"], dtype=np.float32))
    wk = np.ascontiguousarray(np.asarray(inputs["W_key"], dtype=np.float32))
    v = np.ascontiguousarray(np.asarray(inputs["V"], dtype=np.float32))

    Bb, S, H = enc.shape
    dec2 = np.ascontiguousarray(dec.reshape(Bb, H))
    NB = Bb // N_CORES

    nc = get_nc(NB, S, H)
    in_maps = make_in_maps(enc, dec2, wq, wk, v, N_CORES)
    res = run_bass_kernel_spmd(nc, in_maps, list(range(N_CORES))).results
    ctx = np.concatenate([res[i]["ctx"] for i in range(N_CORES)], axis=0)
    attn = np.concatenate([res[i]["attn"] for i in range(N_CORES)], axis=0)
    return (ctx.astype(np.float32), attn.astype(np.float32))


# revision 60
# speedup vs baseline: 1.5011x; 1.5011x over previous
"""Trainium2 Bass kernel for nn_AttentionLayer (Bahdanau-style attention).

reference math (per batch b):
    query  = dec[b] @ W_q                      # [H]
    keys   = enc[b] @ W_k                      # [S, H]
    energy = tanh(query + keys)                # [S, H]
    scores = energy @ V                        # [S]
    attn   = softmax(scores)                   # [S]
    ctx    = attn @ enc[b]                     # [H]

Sharding: data-parallel over batch B=32 across 8 NeuronCores (4 batches/core),
weights replicated, no collectives.

Per-core dataflow (per batch):
  - enc rows are DMA-loaded natural [s,h] fp32 (HWDGE), cast to bf16 on DVE,
    and kept resident for the batch; each 128-row block is also transposed
    SBUF->SBUF by the DMA xbar (dma_start(transpose=True)) into encT tiles
    [128 h_p, 8 h_c, 512 s] so the keys GEMM contracts over h on partitions:
        keysT[j, s] = sum_h W_k[h, j] * encT[h, s]
    with W_k chunks [128h, 128j] stationary and encT [128h, 512s] moving
    -> PSUM [128j, 512s] fp32, accumulated over the 8 h-chunks.
  - tanh(keysT + query) is fused on ScalarE reading PSUM with a per-partition
    bias = queryT[:, j-chunk, b] (query broadcast along the s free dim).
  - scores come from a second matmul: lhsT = V chunk [128,1] bf16, accumulated
    over the 8 j-chunks into PSUM [1, 512].
  - softmax is fp32 on [1, S] rows (reduce_max(negate) -> Exp with bias and
    fused accum_out denominator -> reciprocal -> scale).
  - context = attn @ enc: attn is cast to bf16, partition-scattered and
    xbar-transposed to attnT [128 s, NSC]; the resident bf16 enc rows are the
    moving operand; PSUM accumulates over the 16 s-chunks.

The final module is post-processed by split_excess_waits(): walrus allows only
one sync-wait slot per ISA instruction, so surplus Tile-emitted waits are
hoisted into standalone InstEventSemaphore instructions on the same engine.
"""

import numpy as np
from contextlib import ExitStack

import concourse.bass as bass
import concourse.tile as tile
from concourse import mybir
from concourse.masks import make_identity
from concourse.bass_utils import run_bass_kernel_spmd

F32 = mybir.dt.float32
BF16 = mybir.dt.bfloat16
AF = mybir.ActivationFunctionType
AX = mybir.AxisListType
P = 128

B_FULL, S_FULL, H_FULL = 32, 2048, 1024
N_CORES = 8


def emit(ctx: ExitStack, tc: "tile.TileContext", enc, dec, wq, wk, v,
         ctx_out, attn_out, NB: int, S: int, H: int):
    nc = tc.nc
    HC = H // P            # h-chunks (contraction)
    JC = H // P            # j-chunks (keys output feature dim)
    ST = min(512, S)       # s-tile width (matmul moving dim / PSUM bank)
    NST = S // ST          # s-tiles per batch
    SB = ST // P           # 128-row s-blocks per s-tile
    NSC = S // P           # s-chunks per batch (context contraction)
    CW = min(512, H)       # context psum width
    NCW = H // CW

    const = ctx.enter_context(tc.tile_pool(name="const", bufs=1))
    wkp = ctx.enter_context(tc.tile_pool(name="wkp", bufs=1))
    fload = ctx.enter_context(tc.tile_pool(name="fload", bufs=6))
    enat_p = ctx.enter_context(tc.tile_pool(name="enat", bufs=2 * NSC))
    etp = ctx.enter_context(tc.tile_pool(name="et", bufs=2))
    eng_p = ctx.enter_context(tc.tile_pool(name="energy", bufs=2 * JC + 2))
    small = ctx.enter_context(tc.tile_pool(name="smallsb", bufs=1))
    scores_p = ctx.enter_context(tc.tile_pool(name="scoresp", bufs=2))

    ident_f = const.tile([P, P], F32)
    make_identity(nc, ident_f[:])

    scores_t = {}
    dens_t = {}
    w16_t = {}
    enat_tiles = {}

    def load_stile(b, t):
        """Load+cast one s-tile of enc (HWDGE fp32 + DVE bf16 cast) and
        xbar-transpose it into encT layout."""
        stile_en = []
        for sb in range(SB):
            scn = t * SB + sb
            tf = fload.tile([P, H], F32, tag="fload", name=f"ef{b}_{t}_{sb}")
            s0 = t * ST + sb * P
            nc.sync.dma_start(tf[:], enc[b, s0:s0 + P, :])
            en = enat_p.tile([P, H], BF16, tag="enat", name=f"en{b}_{scn}")
            nc.vector.tensor_copy(en[:], tf[:])
            enat_tiles[(b, scn)] = en
            stile_en.append(en)
        et = etp.tile([P, HC, ST], BF16, tag="et", name=f"et{b}_{t}")
        for sb in range(SB):
            nc.sync.dma_start(et[:, :, sb * P:(sb + 1) * P], stile_en[sb][:],
                              transpose=True)
        return et

    # ---------------- phase 0: weights / query ----------------
    # (scoped PSUM pool: phase-0 banks are released before the main pools)
    wkb = []
    queryT = const.tile([P, JC, NB], F32)
    with tc.tile_pool(name="ph0psum", bufs=1, space="PSUM") as ph0:
        # small inputs first so their DMAs aren't queued behind the weights
        dec_nat = const.tile([NB, H], F32)
        nc.sync.dma_start(dec_nat[:], dec[:, :])
        v_nat = const.tile([1, H], F32)
        nc.sync.dma_start(v_nat[:], v[None, :])
        v_b = const.tile([1, H], BF16)
        nc.vector.tensor_copy(v_b[:], v_nat[:])
        HCp = max(HC, 16)
        v_sc = const.tile([HCp, P], BF16)
        if HCp != HC:
            nc.vector.memset(v_sc[:], 0.0)
        nc.sync.dma_start(v_sc[:HC, :], v_b[:])
        vt_full = const.tile([P, HCp], BF16)
        nc.sync.dma_start(vt_full[:], v_sc[:], transpose=True)
        vt = vt_full[:, :HC]

        # dec -> decT [128 h, HC, NB] f32 (small PE transposes; f32 precision)
        # PE warm-up dummy: observe the gpsimd identity sem with a 1-dep op
        pdum = ph0.tile([P, 4], F32, tag="small", name="pdum")
        nc.tensor.transpose(pdum[:4, :4], ident_f[:4, :4], ident_f[:4, :4])
        decT = const.tile([P, HC, NB], F32)
        for c in range(HC):
            pd = ph0.tile([P, 4], F32, tag="small", name=f"pdec{c}")
            nc.tensor.transpose(pd[:, :NB], dec_nat[:, c * P:(c + 1) * P],
                                ident_f[:NB, :NB])
            nc.vector.tensor_copy(decT[:, c, :], pd[:, :NB])

        # query in natural layout: q_nat[b, j] accumulated over h-chunks with
        # bf16 matmuls (lhsT = decT chunk [128,NB], rhs = W_q rows, fp32
        # accumulation in PSUM), then transposed into queryT [128 j, JC, NB].
        decTb = const.tile([P, HC, NB], BF16)
        nc.vector.tensor_copy(decTb[:], decT[:])
        q_nat_ps = ph0.tile([NB, H], F32, tag="qnat")
        for c in range(HC):
            tf = scores_p.tile([P, H], BF16, tag="wqb", name=f"wqf{c}")
            nc.gpsimd.dma_start(tf[:], wq[c * P:(c + 1) * P, :])
            for half in range(H // CW):
                nc.tensor.matmul(q_nat_ps[:, half * CW:(half + 1) * CW],
                                 decTb[:, c, :], tf[:, half * CW:(half + 1) * CW],
                                 start=(c == 0), stop=(c == HC - 1))
        q_nat = const.tile([NB, H], F32)
        nc.vector.tensor_copy(q_nat[:], q_nat_ps[:])
        for c in range(JC):
            pqt = ph0.tile([P, 4], F32, tag="small", name=f"pqt{c}")
            nc.tensor.transpose(pqt[:, :NB], q_nat[:, c * P:(c + 1) * P],
                                ident_f[:NB, :NB])
            nc.vector.tensor_copy(queryT[:, c, :], pqt[:, :NB])

        # W_k after W_q: needed later (first K-matmul), loads overlap query.
        # SWDGE in-flight cast (gpsimd is idle here; halves DMA write bytes).
        for c in range(HC):
            tb = wkp.tile([P, H], BF16, tag=f"wk{c}")
            nc.gpsimd.dma_start(tb[:], wk[c * P:(c + 1) * P, :])
            wkb.append(tb)

    psum_k = ctx.enter_context(tc.tile_pool(name="psk", bufs=4, space="PSUM"))
    psum_s = ctx.enter_context(tc.tile_pool(name="pss", bufs=2, space="PSUM"))
    psum_c = ctx.enter_context(tc.tile_pool(name="psc", bufs=2, space="PSUM"))

    # ---------------- main loop ----------------
    pending = []  # [(b, t, [energy tiles])]

    def flush_pending():
        # V-matmuls for a finished s-tile, then incremental softmax pieces.
        # No max subtraction: |scores| <= sum|V| (tanh in [-1,1]) stays far
        # below fp32/bf16 exp overflow, so exp(s) is computed per s-tile as
        # soon as its scores land, with per-tile partial denominators.
        while pending:
            b, t, eys = pending.pop(0)
            ps = psum_s.tile([1, ST], F32, tag="ps")
            for jc in range(JC):
                nc.tensor.matmul(ps[:], vt[:, jc:jc + 1], eys[jc][:],
                                 start=(jc == 0), stop=(jc == JC - 1))
            nc.any.tensor_copy(scores_t[b][:, t * ST:(t + 1) * ST], ps[:])
            # incremental per-tile max on DVE (keeps the tail reduction tiny)
            nc.vector.reduce_max(dens_t[b][:, t:t + 1],
                                 scores_t[b][:, t * ST:(t + 1) * ST], axis=AX.X)

    def tail(b):
        # softmax: the 1/denominator scale is deferred -- attnT is built from
        # the unnormalized exp; the scale folds into the context eviction.
        srow = scores_t[b][:]
        mneg = small.tile([1, 1], F32, tag="mneg")
        nc.vector.reduce_max(mneg[:], dens_t[b][:], axis=AX.X, negate=True)
        # bf16 exp straight onto the attnT critical path
        arow_b = small.tile([1, S], BF16, tag="arow_b")
        den = small.tile([1, 1], F32, tag="den")
        nc.scalar.activation(arow_b[:], srow[:], AF.Exp, bias=mneg[:],
                             accum_out=den[:])
        nc.gpsimd.dma_start(w16_t[b][:NSC, :], arow_b[:])
        wT_full = small.tile([P, NSCp], BF16, tag="wT")
        nc.sync.dma_start(wT_full[:], w16_t[b][:], transpose=True)
        wT = wT_full[:, :NSC]

        rec = small.tile([1, 1], F32, tag="rec")
        nc.vector.reciprocal(rec[:], den[:])

        # attn output (normalized, fp32 exp), off the PE critical path
        arow = small.tile([1, S], F32, tag="arow")
        nc.scalar.activation(arow[:], srow[:], AF.Exp, bias=mneg[:])
        arow_n = small.tile([1, S], F32, tag="arow_n")
        nc.vector.tensor_scalar_mul(arow_n[:], arow[:], rec[:])
        nc.sync.dma_start(attn_out[b:b + 1, :], arow_n[:])

        # context: ctx[h] = sum_s attn[s] enc[s, h] over resident bf16 tiles
        pcs = [psum_c.tile([1, CW], F32, tag="pc", name=f"pc{b}_{i}")
               for i in range(NCW)]
        for half in range(NCW):
            for scn in range(NSC):
                nc.tensor.matmul(pcs[half][:], wT[:, scn:scn + 1],
                                 enat_tiles[(b, scn)][:, half * CW:(half + 1) * CW],
                                 start=(scn == 0), stop=(scn == NSC - 1))
        cxt = small.tile([1, H], F32, tag="cxt")
        for half in range(NCW):
            nc.scalar.activation(cxt[:, half * CW:(half + 1) * CW], pcs[half][:],
                                 AF.Copy, scale=rec[:])
        nc.sync.dma_start(ctx_out[b:b + 1, :], cxt[:])

    tail_t = min(2, NST - 1)
    NSCp = max(NSC, 16)
    for b in range(NB):
        scores_t[b] = scores_p.tile([1, S], F32, tag="scores",
                                    name=f"scores{b}")
        dens_t[b] = scores_p.tile([1, NST], F32, tag="dens",
                                  name=f"dens{b}")
        w16_t[b] = scores_p.tile([NSCp, P], BF16, tag="w16",
                                 name=f"w16_{b}")
        if NSCp != NSC:
            nc.vector.memset(w16_t[b][:], 0.0)
        for t in range(NST):
            et = load_stile(b, t)
            # keys GEMM + fused tanh
            eys = []
            for jc in range(JC):
                pk = psum_k.tile([P, ST], F32, tag="pk")
                for c in range(HC):
                    nc.tensor.matmul(pk[:], wkb[c][:, jc * P:(jc + 1) * P],
                                     et[:, c, :], start=(c == 0), stop=(c == HC - 1))
                ey = eng_p.tile([P, ST], BF16, tag="ey")
                nc.scalar.activation(ey[:], pk[:], AF.Tanh,
                                     bias=queryT[:, jc, b:b + 1])
                eys.append(ey)
            # deferred V-matmuls of the previous s-tile (keeps PE dense)
            flush_pending()
            pending.append((b, t, eys))
            if t == tail_t and b > 0:
                tail(b - 1)
    flush_pending()
    tail(NB - 1)


_NO_SPLIT = {"InstEventSemaphore", "InstNoOp",
             "InstUnconditionalBranch", "InstHalt"}


def split_excess_waits(nc) -> int:
    """Walrus ISA instruction encodings have a single sync-wait slot. Tile can
    emit more. Hoist the excess into standalone InstEventSemaphore
    instructions on the same engine, immediately before the instruction."""
    n_split = 0
    for f in nc.m.functions:
        for blk in f.blocks:
            insts = blk.instructions
            out = []
            for inst in insts:
                ty = type(inst).__name__
                si = inst.sync_info
                if si is None or ty in _NO_SPLIT:
                    out.append(inst)
                    continue
                limit = 1
                waits = list(si.on_wait)
                if len(waits) > limit:
                    excess, keep = waits[:-limit], waits[-limit:]
                    for w in excess:
                        n_split += 1
                        ev = mybir.InstEventSemaphore(
                            name=f"I-evw{n_split}", ins=[], outs=[],
                            engine=inst.engine,
                            sync_info=mybir.SyncInfo(on_wait=[w], on_update=[]),
                            bass_nofuse=True,
                        )
                        out.append(ev)
                    inst.sync_info = mybir.SyncInfo(on_wait=keep,
                                                   on_update=list(si.on_update))
                out.append(inst)
            insts[:] = out
    return n_split


def build_nc(NB: int, S: int, H: int, split_waits: bool = True) -> bass.Bass:
    nc = bass.Bass("TRN2", target_bir_lowering=False, debug=False)
    enc = nc.dram_tensor("enc", [NB, S, H], F32, kind="ExternalInput").ap()
    dec = nc.dram_tensor("dec", [NB, H], F32, kind="ExternalInput").ap()
    wq = nc.dram_tensor("wq", [H, H], F32, kind="ExternalInput").ap()
    wk = nc.dram_tensor("wk", [H, H], F32, kind="ExternalInput").ap()
    v = nc.dram_tensor("v", [H], F32, kind="ExternalInput").ap()
    ctx_out = nc.dram_tensor("ctx", [NB, H], F32, kind="ExternalOutput").ap()
    attn_out = nc.dram_tensor("attn", [NB, S], F32, kind="ExternalOutput").ap()
    with tile.TileContext(nc) as tc, ExitStack() as ctx:
        emit(ctx, tc, enc, dec, wq, wk, v, ctx_out, attn_out, NB, S, H)
    if split_waits:
        split_excess_waits(nc)
    return nc


_NC_CACHE = {}


def get_nc(NB: int, S: int, H: int) -> bass.Bass:
    key = (NB, S, H)
    if key not in _NC_CACHE:
        _NC_CACHE[key] = build_nc(NB, S, H)
    return _NC_CACHE[key]


def make_in_maps(enc, dec, wq, wk, v, n_cores):
    nb = enc.shape[0] // n_cores
    return [
        {
            "enc": np.ascontiguousarray(enc[i * nb:(i + 1) * nb]),
            "dec": np.ascontiguousarray(dec[i * nb:(i + 1) * nb]),
            "wq": wq,
            "wk": wk,
            "v": v,
        }
        for i in range(n_cores)
    ]


def kernel(**inputs) -> tuple:
    enc = np.asarray(inputs["encoder_outputs"], dtype=np.float32)
    dec = np.asarray(inputs["decoder_hidden"], dtype=np.float32)
    wq = np.ascontiguousarray(np.asarray(inputs["W_query"], dtype=np.float32))
    wk = np.ascontiguousarray(np.asarray(inputs["W_key"], dtype=np.float32))
    v = np.ascontiguousarray(np.asarray(inputs["V"], dtype=np.float32))

    Bb, S, H = enc.shape
    dec2 = np.ascontiguousarray(dec.reshape(Bb, H))
    NB = Bb // N_CORES

    nc = get_nc(NB, S, H)
    in_maps = make_in_maps(enc, dec2, wq, wk, v, N_CORES)
    res = run_bass_kernel_spmd(nc, in_maps, list(range(N_CORES))).results
    ctx = np.concatenate([res[i]["ctx"] for i in range(N_CORES)], axis=0)
    attn = np.concatenate([res[i]["attn"] for i in range(N_CORES)], axis=0)
    return (ctx.astype(np.float32), attn.astype(np.float32))


# revision 61
# speedup vs baseline: 5.3118x; 3.5385x over previous
"""Trainium2 Bass kernel for nn_AttentionLayer (Bahdanau-style attention).

reference math (per batch b):
    query  = dec[b] @ W_q                      # [H]
    keys   = enc[b] @ W_k                      # [S, H]
    energy = tanh(query + keys)                # [S, H]
    scores = energy @ V                        # [S]
    attn   = softmax(scores)                   # [S]
    ctx    = attn @ enc[b]                     # [H]

Sharding: data-parallel over batch B=32 across 8 NeuronCores (4 batches/core),
weights replicated, no collectives.

Per-core dataflow (per batch):
  - enc rows are DMA-loaded natural [s,h] fp32 (HWDGE), cast to bf16 on DVE,
    and kept resident for the batch; each 128-row block is also transposed
    SBUF->SBUF by the DMA xbar (dma_start(transpose=True)) into encT tiles
    [128 h_p, 8 h_c, 512 s] so the keys GEMM contracts over h on partitions:
        keysT[j, s] = sum_h W_k[h, j] * encT[h, s]
    with W_k chunks [128h, 128j] stationary and encT [128h, 512s] moving
    -> PSUM [128j, 512s] fp32, accumulated over the 8 h-chunks.
  - tanh(keysT + query) is fused on ScalarE reading PSUM with a per-partition
    bias = queryT[:, j-chunk, b] (query broadcast along the s free dim).
  - scores come from a second matmul: lhsT = V chunk [128,1] bf16, accumulated
    over the 8 j-chunks into PSUM [1, 512].
  - softmax is fp32 on [1, S] rows (reduce_max(negate) -> Exp with bias and
    fused accum_out denominator -> reciprocal -> scale).
  - context = attn @ enc: attn is cast to bf16, partition-scattered and
    xbar-transposed to attnT [128 s, NSC]; the resident bf16 enc rows are the
    moving operand; PSUM accumulates over the 16 s-chunks.

The final module is post-processed by split_excess_waits(): walrus allows only
one sync-wait slot per ISA instruction, so surplus Tile-emitted waits are
hoisted into standalone InstEventSemaphore instructions on the same engine.
"""

import numpy as np
from contextlib import ExitStack

import concourse.bass as bass
import concourse.tile as tile
from concourse import mybir
from concourse.masks import make_identity
from concourse.bass_utils import run_bass_kernel_spmd

F32 = mybir.dt.float32
BF16 = mybir.dt.bfloat16
AF = mybir.ActivationFunctionType
AX = mybir.AxisListType
P = 128

B_FULL, S_FULL, H_FULL = 32, 2048, 1024
N_CORES = 8


def emit(ctx: ExitStack, tc: "tile.TileContext", enc, dec, wq, wk, v,
         ctx_out, attn_out, NB: int, S: int, H: int):
    nc = tc.nc
    HC = H // P            # h-chunks (contraction)
    JC = H // P            # j-chunks (keys output feature dim)
    ST = min(512, S)       # s-tile width (matmul moving dim / PSUM bank)
    NST = S // ST          # s-tiles per batch
    SB = ST // P           # 128-row s-blocks per s-tile
    NSC = S // P           # s-chunks per batch (context contraction)
    CW = min(512, H)       # context psum width
    NCW = H // CW

    const = ctx.enter_context(tc.tile_pool(name="const", bufs=1))
    wkp = ctx.enter_context(tc.tile_pool(name="wkp", bufs=1))
    fload = ctx.enter_context(tc.tile_pool(name="fload", bufs=6))
    enat_p = ctx.enter_context(tc.tile_pool(name="enat", bufs=2 * NSC))
    etp = ctx.enter_context(tc.tile_pool(name="et", bufs=2))
    eng_p = ctx.enter_context(tc.tile_pool(name="energy", bufs=2 * JC + 2))
    small = ctx.enter_context(tc.tile_pool(name="smallsb", bufs=1))
    scores_p = ctx.enter_context(tc.tile_pool(name="scoresp", bufs=2))

    ident_f = const.tile([P, P], F32)
    make_identity(nc, ident_f[:])

    scores_t = {}
    dens_t = {}
    w16_t = {}
    enat_tiles = {}

    def load_stile(b, t):
        """Load+cast one s-tile of enc (HWDGE fp32 + DVE bf16 cast) and
        xbar-transpose it into encT layout."""
        stile_en = []
        for sb in range(SB):
            scn = t * SB + sb
            tf = fload.tile([P, H], F32, tag="fload", name=f"ef{b}_{t}_{sb}")
            s0 = t * ST + sb * P
            nc.sync.dma_start(tf[:], enc[b, s0:s0 + P, :])
            en = enat_p.tile([P, H], BF16, tag="enat", name=f"en{b}_{scn}")
            nc.vector.tensor_copy(en[:], tf[:])
            enat_tiles[(b, scn)] = en
            stile_en.append(en)
        et = etp.tile([P, HC, ST], BF16, tag="et", name=f"et{b}_{t}")
        for sb in range(SB):
            nc.sync.dma_start(et[:, :, sb * P:(sb + 1) * P], stile_en[sb][:],
                              transpose=True)
        return et

    # ---------------- phase 0: weights / query ----------------
    # (scoped PSUM pool: phase-0 banks are released before the main pools)
    wkb = []
    queryT = const.tile([P, JC, NB], F32)
    with tc.tile_pool(name="ph0psum", bufs=1, space="PSUM") as ph0:
        # small inputs first so their DMAs aren't queued behind the weights
        dec_nat = const.tile([NB, H], F32)
        nc.sync.dma_start(dec_nat[:], dec[:, :])
        v_nat = const.tile([1, H], F32)
        nc.sync.dma_start(v_nat[:], v[None, :])
        v_b = const.tile([1, H], BF16)
        nc.vector.tensor_copy(v_b[:], v_nat[:])
        HCp = max(HC, 16)
        v_sc = const.tile([HCp, P], BF16)
        if HCp != HC:
            nc.vector.memset(v_sc[:], 0.0)
        nc.sync.dma_start(v_sc[:HC, :], v_b[:])
        vt_full = const.tile([P, HCp], BF16)
        nc.sync.dma_start(vt_full[:], v_sc[:], transpose=True)
        vt = vt_full[:, :HC]

        # dec -> decT [128 h, HC, NB] f32 (small PE transposes; f32 precision)
        # PE warm-up dummy: observe the gpsimd identity sem with a 1-dep op
        pdum = ph0.tile([P, 4], F32, tag="small", name="pdum")
        nc.tensor.transpose(pdum[:4, :4], ident_f[:4, :4], ident_f[:4, :4])
        decT = const.tile([P, HC, NB], F32)
        for c in range(HC):
            pd = ph0.tile([P, 4], F32, tag="small", name=f"pdec{c}")
            nc.tensor.transpose(pd[:, :NB], dec_nat[:, c * P:(c + 1) * P],
                                ident_f[:NB, :NB])
            nc.vector.tensor_copy(decT[:, c, :], pd[:, :NB])

        # query in natural layout: q_nat[b, j] accumulated over h-chunks with
        # bf16 matmuls (lhsT = decT chunk [128,NB], rhs = W_q rows, fp32
        # accumulation in PSUM), then transposed into queryT [128 j, JC, NB].
        decTb = const.tile([P, HC, NB], BF16)
        nc.vector.tensor_copy(decTb[:], decT[:])
        q_nat_ps = ph0.tile([NB, H], F32, tag="qnat")
        for c in range(HC):
            tf = scores_p.tile([P, H], BF16, tag="wqb", name=f"wqf{c}")
            nc.gpsimd.dma_start(tf[:], wq[c * P:(c + 1) * P, :])
            for half in range(H // CW):
                nc.tensor.matmul(q_nat_ps[:, half * CW:(half + 1) * CW],
                                 decTb[:, c, :], tf[:, half * CW:(half + 1) * CW],
                                 start=(c == 0), stop=(c == HC - 1))
        q_nat = const.tile([NB, H], F32)
        nc.vector.tensor_copy(q_nat[:], q_nat_ps[:])
        for c in range(JC):
            pqt = ph0.tile([P, 4], F32, tag="small", name=f"pqt{c}")
            nc.tensor.transpose(pqt[:, :NB], q_nat[:, c * P:(c + 1) * P],
                                ident_f[:NB, :NB])
            nc.vector.tensor_copy(queryT[:, c, :], pqt[:, :NB])

        # W_k after W_q: needed later (first K-matmul), loads overlap query.
        # SWDGE in-flight cast (gpsimd is idle here; halves DMA write bytes).
        for c in range(HC):
            tb = wkp.tile([P, H], BF16, tag=f"wk{c}")
            nc.gpsimd.dma_start(tb[:], wk[c * P:(c + 1) * P, :])
            wkb.append(tb)

    psum_k = ctx.enter_context(tc.tile_pool(name="psk", bufs=4, space="PSUM"))
    psum_s = ctx.enter_context(tc.tile_pool(name="pss", bufs=2, space="PSUM"))
    psum_c = ctx.enter_context(tc.tile_pool(name="psc", bufs=2, space="PSUM"))

    # ---------------- main loop ----------------
    pending = []  # [(b, t, [energy tiles])]

    def flush_pending():
        # V-matmuls for a finished s-tile, then incremental softmax pieces.
        # No max subtraction: |scores| <= sum|V| (tanh in [-1,1]) stays far
        # below fp32/bf16 exp overflow, so exp(s) is computed per s-tile as
        # soon as its scores land, with per-tile partial denominators.
        while pending:
            b, t, eys = pending.pop(0)
            ps = psum_s.tile([1, ST], F32, tag="ps")
            for jc in range(JC):
                nc.tensor.matmul(ps[:], vt[:, jc:jc + 1], eys[jc][:],
                                 start=(jc == 0), stop=(jc == JC - 1))
            nc.vector.tensor_copy(scores_t[b][:, t * ST:(t + 1) * ST], ps[:])
            # incremental per-tile max on DVE (keeps the tail reduction tiny)
            nc.vector.reduce_max(dens_t[b][:, t:t + 1],
                                 scores_t[b][:, t * ST:(t + 1) * ST], axis=AX.X)

    def tail(b):
        # softmax: the 1/denominator scale is deferred -- attnT is built from
        # the unnormalized exp; the scale folds into the context eviction.
        srow = scores_t[b][:]
        mneg = small.tile([1, 1], F32, tag="mneg")
        nc.vector.reduce_max(mneg[:], dens_t[b][:], axis=AX.X, negate=True)
        # bf16 exp straight onto the attnT critical path
        arow_b = small.tile([1, S], BF16, tag="arow_b")
        den = small.tile([1, 1], F32, tag="den")
        nc.scalar.activation(arow_b[:], srow[:], AF.Exp, bias=mneg[:],
                             accum_out=den[:])
        nc.gpsimd.dma_start(w16_t[b][:NSC, :], arow_b[:])
        wT_full = small.tile([P, NSCp], BF16, tag="wT")
        nc.sync.dma_start(wT_full[:], w16_t[b][:], transpose=True)
        wT = wT_full[:, :NSC]

        rec = small.tile([1, 1], F32, tag="rec")
        nc.vector.reciprocal(rec[:], den[:])

        # attn output (normalized, fp32 exp), off the PE critical path
        arow = small.tile([1, S], F32, tag="arow")
        nc.scalar.activation(arow[:], srow[:], AF.Exp, bias=mneg[:])
        arow_n = small.tile([1, S], F32, tag="arow_n")
        nc.vector.tensor_scalar_mul(arow_n[:], arow[:], rec[:])
        nc.sync.dma_start(attn_out[b:b + 1, :], arow_n[:])

        # context: ctx[h] = sum_s attn[s] enc[s, h] over resident bf16 tiles
        pcs = [psum_c.tile([1, CW], F32, tag="pc", name=f"pc{b}_{i}")
               for i in range(NCW)]
        for half in range(NCW):
            for scn in range(NSC):
                nc.tensor.matmul(pcs[half][:], wT[:, scn:scn + 1],
                                 enat_tiles[(b, scn)][:, half * CW:(half + 1) * CW],
                                 start=(scn == 0), stop=(scn == NSC - 1))
        cxt = small.tile([1, H], F32, tag="cxt")
        for half in range(NCW):
            nc.scalar.activation(cxt[:, half * CW:(half + 1) * CW], pcs[half][:],
                                 AF.Copy, scale=rec[:])
        nc.sync.dma_start(ctx_out[b:b + 1, :], cxt[:])

    tail_t = min(2, NST - 1)
    NSCp = max(NSC, 16)
    for b in range(NB):
        scores_t[b] = scores_p.tile([1, S], F32, tag="scores",
                                    name=f"scores{b}")
        dens_t[b] = scores_p.tile([1, NST], F32, tag="dens",
                                  name=f"dens{b}")
        w16_t[b] = scores_p.tile([NSCp, P], BF16, tag="w16",
                                 name=f"w16_{b}")
        if NSCp != NSC:
            nc.vector.memset(w16_t[b][:], 0.0)
        for t in range(NST):
            et = load_stile(b, t)
            # keys GEMM + fused tanh
            eys = []
            for jc in range(JC):
                pk = psum_k.tile([P, ST], F32, tag="pk")
                for c in range(HC):
                    nc.tensor.matmul(pk[:], wkb[c][:, jc * P:(jc + 1) * P],
                                     et[:, c, :], start=(c == 0), stop=(c == HC - 1))
                ey = eng_p.tile([P, ST], BF16, tag="ey")
                nc.scalar.activation(ey[:], pk[:], AF.Tanh,
                                     bias=queryT[:, jc, b:b + 1])
                eys.append(ey)
            # deferred V-matmuls of the previous s-tile (keeps PE dense)
            flush_pending()
            pending.append((b, t, eys))
            if t == tail_t and b > 0:
                tail(b - 1)
    flush_pending()
    tail(NB - 1)


_NO_SPLIT = {"InstEventSemaphore", "InstNoOp",
             "InstUnconditionalBranch", "InstHalt"}


def split_excess_waits(nc) -> int:
    """Walrus ISA instruction encodings have a single sync-wait slot. Tile can
    emit more. Hoist the excess into standalone InstEventSemaphore
    instructions on the same engine, immediately before the instruction."""
    n_split = 0
    for f in nc.m.functions:
        for blk in f.blocks:
            insts = blk.instructions
            out = []
            for inst in insts:
                ty = type(inst).__name__
                si = inst.sync_info
                if si is None or ty in _NO_SPLIT:
                    out.append(inst)
                    continue
                limit = 1
                waits = list(si.on_wait)
                if len(waits) > limit:
                    excess, keep = waits[:-limit], waits[-limit:]
                    for w in excess:
                        n_split += 1
                        ev = mybir.InstEventSemaphore(
                            name=f"I-evw{n_split}", ins=[], outs=[],
                            engine=inst.engine,
                            sync_info=mybir.SyncInfo(on_wait=[w], on_update=[]),
                            bass_nofuse=True,
                        )
                        out.append(ev)
                    inst.sync_info = mybir.SyncInfo(on_wait=keep,
                                                   on_update=list(si.on_update))
                out.append(inst)
            insts[:] = out
    return n_split


def build_nc(NB: int, S: int, H: int, split_waits: bool = True) -> bass.Bass:
    nc = bass.Bass("TRN2", target_bir_lowering=False, debug=False)
    enc = nc.dram_tensor("enc", [NB, S, H], F32, kind="ExternalInput").ap()
    dec = nc.dram_tensor("dec", [NB, H], F32, kind="ExternalInput").ap()
    wq = nc.dram_tensor("wq", [H, H], F32, kind="ExternalInput").ap()
    wk = nc.dram_tensor("wk", [H, H], F32, kind="ExternalInput").ap()
    v = nc.dram_tensor("v", [H], F32, kind="ExternalInput").ap()
    ctx_out = nc.dram_tensor("ctx", [NB, H], F32, kind="ExternalOutput").ap()
    attn_out = nc.dram_tensor("attn", [NB, S], F32, kind="ExternalOutput").ap()
    with tile.TileContext(nc) as tc, ExitStack() as ctx:
        emit(ctx, tc, enc, dec, wq, wk, v, ctx_out, attn_out, NB, S, H)
    if split_waits:
        split_excess_waits(nc)
    return nc


_NC_CACHE = {}


def get_nc(NB: int, S: int, H: int) -> bass.Bass:
    key = (NB, S, H)
    if key not in _NC_CACHE:
        _NC_CACHE[key] = build_nc(NB, S, H)
    return _NC_CACHE[key]


def make_in_maps(enc, dec, wq, wk, v, n_cores):
    nb = enc.shape[0] // n_cores
    return [
        {
            "enc": np.ascontiguousarray(enc[i * nb:(i + 1) * nb]),
            "dec": np.ascontiguousarray(dec[i * nb:(i + 1) * nb]),
            "wq": wq,
            "wk": wk,
            "v": v,
        }
        for i in range(n_cores)
    ]


def kernel(**inputs) -> tuple:
    enc = np.asarray(inputs["encoder_outputs"], dtype=np.float32)
    dec = np.asarray(inputs["decoder_hidden"], dtype=np.float32)
    wq = np.ascontiguousarray(np.asarray(inputs["W_query"], dtype=np.float32))
    wk = np.ascontiguousarray(np.asarray(inputs["W_key"], dtype=np.float32))
    v = np.ascontiguousarray(np.asarray(inputs["V"], dtype=np.float32))

    Bb, S, H = enc.shape
    dec2 = np.ascontiguousarray(dec.reshape(Bb, H))
    NB = Bb // N_CORES

    nc = get_nc(NB, S, H)
    in_maps = make_in_maps(enc, dec2, wq, wk, v, N_CORES)
    res = run_bass_kernel_spmd(nc, in_maps, list(range(N_CORES))).results
    ctx = np.concatenate([res[i]["ctx"] for i in range(N_CORES)], axis=0)
    attn = np.concatenate([res[i]["attn"] for i in range(N_CORES)], axis=0)
    return (ctx.astype(np.float32), attn.astype(np.float32))
